# revision 4
# baseline (speedup 1.0000x reference)
"""Trainium2 Bass kernel for a GCN layer:

    out = segment_sum(support[edge_col] * edge_val, edge_row) + bias
    where support = vertex @ W

Strategy (8 NeuronCores, SPMD):
  - Destination nodes row-partitioned: core c owns rows [c*12500, (c+1)*12500),
    98 destination windows of 128 rows each.
  - Edges are grouped host-side by (dest window w, source range r) where r is
    one of 4 ranges of 25000 source nodes (so the in-range row index fits the
    int16 indices of the SWDGE dma_gather ucode).  Chunk = 128 edges of one
    (w, r) group; chunk counts per (w, r) are equalized across cores (max) so
    one SPMD program serves all 8 cores.
  - Windows are processed in batches of WB=8.  Per (batch, range): ONE
    dma_gather instruction fetches all the batch's source rows for that range
    (thousands of indices per instruction - the 994ns SWDGE fixed overhead is
    amortized), in bf16 (256B/row).
  - Per chunk: a val-weighted one-hot matrix oh[e, d] = (dloc[e]==d)*val[e]
    is built in one fused tensor_scalar (bf16, split between DVE and Pool
    engines), then matmul aggT[feat, dest] += gathered[e, feat].T @ oh[e, d]
    accumulates in PSUM (bf16 matmul: 1 cycle/row).
  - Per window: out[d, :] = (aggT.T @ W) + bias.  Aggregation happens in
    input-feature space so no support table and no collective is needed.
"""

import numpy as np
import ml_dtypes

N_NODES = 100000
N_EDGES = 1600000
IN_F = 128
OUT_F = 64
N_CORES = 8
P = 128
NR = 4                     # source ranges (int16 idx limit: rsz <= 32768)
WB = 8                     # dest windows per batch
MAXCH = 64                 # max chunks per dma_gather piece (8192 idxs)
OH_DVE = 7                 # of 10: share of oh builds on DVE (rest on Pool)


def _cdiv(a, b):
    return -(-a // b)


def _preprocess(edge_row, edge_col, edge_val, n_nodes=N_NODES,
                n_cores=N_CORES, wb=WB, nr=NR):
    """Group edges by (core, window, src-range); equalize chunk counts across
    cores; build per-core idx/dloc/val tables and the shared program plan."""
    n_shard = n_nodes // n_cores
    n_win = _cdiv(n_shard, P)
    rsz = _cdiv(n_nodes, nr)
    nb = _cdiv(n_win, wb)

    rows = np.asarray(edge_row, np.int64)
    cols = np.asarray(edge_col, np.int64)
    vals = np.asarray(edge_val, np.float32)

    core = rows // n_shard
    local = rows - core * n_shard
    win = local // P
    dloc = (local % P).astype(np.float32)
    rng_ = cols // rsz
    idx16 = (cols - rng_ * rsz).astype(np.int16)

    cnt = np.zeros((n_cores, n_win, nr), np.int64)
    np.add.at(cnt, (core, win, rng_), 1)
    nch_wr = _cdiv(cnt.max(axis=0), P)             # [n_win, nr]
    empty_w = nch_wr.sum(axis=1) == 0
    nch_wr[empty_w, 0] = 1                         # every window needs >=1

    # enumerate chunks in (batch, range, window, k) order; build gather pieces
    cid_base = np.zeros((n_win, nr), np.int64)
    gathers = []                                   # (batch, range, cid0, nch)
    cid = 0
    for b in range(nb):
        ws = list(range(b * wb, min(n_win, (b + 1) * wb)))
        for r in range(nr):
            total = int(nch_wr[ws, r].sum())
            if total == 0:
                continue
            cc = cid
            for w in ws:
                cid_base[w, r] = cc
                cc += int(nch_wr[w, r])
            off = 0
            while off < total:
                n = min(MAXCH, total - off)
                gathers.append((b, r, cid + off, n))
                off += n
            cid += total
    nch_total = cid
    nidx = nch_total * P

    piece_of = {}
    for gi, (b, r, cb, n) in enumerate(gathers):
        for k in range(n):
            piece_of[cb + k] = (gi, k)
    windows = []                                   # [w] -> [(gi, slab, cid)]
    for w in range(n_win):
        lst = []
        for r in range(nr):
            for k in range(int(nch_wr[w, r])):
                c = int(cid_base[w, r]) + k
                gi, slab = piece_of[c]
                lst.append((gi, slab, c))
        windows.append(lst)

    # per-core tables
    idx_flat = np.zeros((n_cores, nidx), np.int16)
    dloc_tab = np.zeros((n_cores, P, nch_total), np.float32)
    vals_tab = np.zeros((n_cores, P, nch_total), np.float32)

    key = (core * n_win + win) * nr + rng_
    order = np.argsort(key, kind="stable")
    ks = key[order]
    starts = np.r_[0, np.nonzero(np.diff(ks))[0] + 1]
    gid = np.zeros(len(ks), np.int64)
    gid[starts[1:]] = 1
    gid = np.cumsum(gid)
    rank = np.arange(len(ks)) - starts[gid]
    ec, ew, er = core[order], win[order], rng_[order]
    chunk_k = rank // P
    part = rank % P
    cidv = cid_base[ew, er] + chunk_k
    dloc_tab[ec, part, cidv] = dloc[order]
    vals_tab[ec, part, cidv] = vals[order]
    idx_flat[ec, cidv * P + part] = idx16[order]

    wrapped = idx_flat.reshape(n_cores, nidx // 16, 16).transpose(0, 2, 1)
    idx_tab = np.ascontiguousarray(np.tile(wrapped, (1, 8, 1)))  # [c,128,cols]

    plan = dict(n_nodes=n_nodes, n_cores=n_cores, n_shard=n_shard,
                n_win=n_win, rsz=rsz, nb=nb, wb=wb, nr=nr,
                nch_total=nch_total, nidx=nidx,
                gathers=gathers, windows=windows)
    return plan, idx_tab, dloc_tab, vals_tab


def _build_nc(plan, in_f=IN_F, out_f=OUT_F):
    import concourse.bacc as bacc
    import concourse.mybir as mybir
    import concourse.tile as tile

    f32 = mybir.dt.float32
    bf16 = mybir.dt.bfloat16
    i16 = mybir.dt.int16
    nc = bacc.Bacc("TRN2", target_bir_lowering=False, debug=False,
                   enable_asserts=False)

    n_nodes = plan["n_nodes"]
    n_shard = plan["n_shard"]
    n_win = plan["n_win"]
    rsz = plan["rsz"]
    nb = plan["nb"]
    wb = plan["wb"]
    nch_total = plan["nch_total"]
    nidx = plan["nidx"]
    gathers = plan["gathers"]
    windows = plan["windows"]

    maxnch = max(g[3] for g in gathers)
    pieces_by_batch = [[] for _ in range(nb)]
    for gi, g in enumerate(gathers):
        pieces_by_batch[g[0]].append(gi)
    gbufs = max(len(pieces_by_batch[b]) + len(pieces_by_batch[b + 1])
                for b in range(nb - 1)) if nb > 1 else len(pieces_by_batch[0])

    vertex = nc.dram_tensor("vertex", [n_nodes, in_f], bf16,
                            kind="ExternalInput").ap()
    wmat = nc.dram_tensor("wmat", [in_f, out_f], bf16,
                          kind="ExternalInput").ap()
    bias_rep = nc.dram_tensor("bias_rep", [P, out_f], f32,
                              kind="ExternalInput").ap()
    iota = nc.dram_tensor("iota", [P, P], bf16, kind="ExternalInput").ap()
    idxs = nc.dram_tensor("idxs", [P, nidx // 16], i16,
                          kind="ExternalInput").ap()
    dloc = nc.dram_tensor("dloc", [P, nch_total], f32,
                          kind="ExternalInput").ap()
    vals = nc.dram_tensor("vals", [P, nch_total], f32,
                          kind="ExternalInput").ap()
    out = nc.dram_tensor("out", [n_shard, out_f], f32,
                         kind="ExternalOutput").ap()

    with tile.TileContext(nc) as tc:
        with (
            tc.tile_pool(name="const", bufs=1) as cpool,
            tc.tile_pool(name="meta", bufs=1) as mpool,
            tc.tile_pool(name="gather", bufs=gbufs) as gpool,
            tc.tile_pool(name="oh", bufs=12) as opool,
            tc.tile_pool(name="evac", bufs=4) as epool,
            tc.tile_pool(name="osb", bufs=4) as spool,
            tc.tile_pool(name="agg_psum", bufs=4, space="PSUM") as agg_pp,
            tc.tile_pool(name="out_psum", bufs=2, space="PSUM") as out_pp,
        ):
            w_sb = cpool.tile([in_f, out_f], bf16)
            nc.sync.dma_start(out=w_sb[:], in_=wmat[:])
            bias_sb = cpool.tile([P, out_f], f32)
            nc.sync.dma_start(out=bias_sb[:], in_=bias_rep[:])
            iota_sb = cpool.tile([P, P], bf16)
            nc.sync.dma_start(out=iota_sb[:], in_=iota[:])
            idx_sb = mpool.tile([P, nidx // 16], i16)
            nc.sync.dma_start(out=idx_sb[:], in_=idxs[:])
            dloc_sb = mpool.tile([P, nch_total], f32)
            nc.sync.dma_start(out=dloc_sb[:], in_=dloc[:])
            vals_sb = mpool.tile([P, nch_total], f32)
            nc.sync.dma_start(out=vals_sb[:], in_=vals[:])

            tiles = {}

            def emit_gathers(b):
                for gi in pieces_by_batch[b]:
                    _, r, cb, n = gathers[gi]
                    t = gpool.tile([P, maxnch * in_f], bf16, tag="g")
                    nc.gpsimd.dma_gather(
                        out_ap=t[:, :n * in_f].rearrange(
                            "p (g f) -> p g f", f=in_f),
                        in_ap=vertex[r * rsz:n_nodes, :],
                        idxs_ap=idx_sb[:, cb * 8:(cb + n) * 8],
                        num_idxs=n * P,
                        num_idxs_reg=n * P,
                        elem_size=in_f,
                        single_packet=False,
                    )
                    tiles[gi] = t

            emit_gathers(0)
            if nb > 1:
                emit_gathers(1)
            for b in range(nb):
                for w in range(b * wb, min(n_win, (b + 1) * wb)):
                    chunks = windows[w]
                    m = len(chunks)
                    aggT = agg_pp.tile([in_f, P], f32, tag="agg")
                    for j, (gi, slab, c) in enumerate(chunks):
                        oh = opool.tile([P, P], bf16, tag="oh")
                        eng = nc.vector if (c % 10) < OH_DVE else nc.gpsimd
                        eng.tensor_scalar(
                            out=oh[:],
                            in0=iota_sb[:],
                            scalar1=dloc_sb[:, c:c + 1],
                            scalar2=vals_sb[:, c:c + 1],
                            op0=mybir.AluOpType.is_equal,
                            op1=mybir.AluOpType.mult,
                        )
                        nc.tensor.matmul(
                            out=aggT[:],
                            lhsT=tiles[gi][:, slab * in_f:(slab + 1) * in_f],
                            rhs=oh[:],
                            start=(j == 0),
                            stop=(j == m - 1),
                        )
                    aggT_sb = epool.tile([in_f, P], bf16, tag="ev")
                    nc.scalar.activation(
                        out=aggT_sb[:], in_=aggT[:],
                        func=mybir.ActivationFunctionType.Copy)
                    outw = out_pp.tile([P, out_f], f32, tag="ow")
                    nc.tensor.matmul(out=outw[:], lhsT=aggT_sb[:],
                                     rhs=w_sb[:], start=True, stop=True)
                    out_sb = spool.tile([P, out_f], f32, tag="os")
                    nc.vector.tensor_tensor(out=out_sb[:], in0=outw[:],
                                            in1=bias_sb[:],
                                            op=mybir.AluOpType.add)
                    rows = min(P, n_shard - w * P)
                    nc.sync.dma_start(out=out[w * P:w * P + rows, :],
                                      in_=out_sb[:rows, :])
                if b + 2 < nb:
                    emit_gathers(b + 2)

    nc.compile()
    return nc


def _make_in_maps(vertex, weights, bias, idx_tab, dloc_tab, vals_tab,
                  n_cores=N_CORES):
    iota = np.tile(np.arange(P, dtype=ml_dtypes.bfloat16)[None, :], (P, 1))
    bias_rep = np.ascontiguousarray(
        np.tile(np.asarray(bias, np.float32)[None, :], (P, 1)))
    vertex_bf = np.ascontiguousarray(
        np.asarray(vertex, np.float32).astype(ml_dtypes.bfloat16))
    w_bf = np.ascontiguousarray(
        np.asarray(weights, np.float32).astype(ml_dtypes.bfloat16))
    return [
        {
            "vertex": vertex_bf,
            "wmat": w_bf,
            "bias_rep": bias_rep,
            "iota": iota,
            "idxs": idx_tab[c],
            "dloc": dloc_tab[c],
            "vals": vals_tab[c],
        }
        for c in range(n_cores)
    ]


def _run(nc, in_maps, trace=False, tmpdir=None):
    from concourse import bass_utils
    from concourse.bass_interp import get_hw_module

    old_m = nc.m
    nc.m = get_hw_module(nc.m)
    try:
        return bass_utils.run_bass_kernel_spmd(
            nc, in_maps, core_ids=list(range(len(in_maps))),
            trace=trace, tmpdir=tmpdir)
    finally:
        nc.m = old_m


def kernel(**inputs):
    vertex = np.asarray(inputs["vertex"], dtype=np.float32)
    edge_row = np.asarray(inputs["edge_row"])
    edge_col = np.asarray(inputs["edge_col"])
    edge_val = np.asarray(inputs["edge_val"], dtype=np.float32)
    weights = np.asarray(inputs["weights"], dtype=np.float32)
    bias = np.asarray(inputs["bias"], dtype=np.float32)

    plan, idx_tab, dloc_tab, vals_tab = _preprocess(
        edge_row, edge_col, edge_val)
    nc = _build_nc(plan)
    in_maps = _make_in_maps(vertex, weights, bias, idx_tab, dloc_tab,
                            vals_tab)
    res = _run(nc, in_maps)
    return np.concatenate([res.results[c]["out"] for c in range(N_CORES)],
                          axis=0)


# revision 5
# speedup vs baseline: 2.5615x; 2.5615x over previous
"""Trainium2 Bass kernel for a GCN layer:

    out = segment_sum(support[edge_col] * edge_val, edge_row) + bias
    where support = vertex @ W

Strategy (8 NeuronCores, SPMD):
  - Destination nodes row-partitioned: core c owns rows [c*12500, (c+1)*12500),
    98 destination windows of 128 rows each.
  - Edges are grouped host-side by (dest window w, source range r) where r is
    one of 4 ranges of 25000 source nodes (so the in-range row index fits the
    int16 indices of the SWDGE dma_gather ucode).  Chunk = 128 edges of one
    (w, r) group; chunk counts per (w, r) are equalized across cores (max) so
    one SPMD program serves all 8 cores.
  - Windows are processed in batches of WB=8.  Per (batch, range): ONE
    dma_gather instruction fetches all the batch's source rows for that range
    (thousands of indices per instruction - the 994ns SWDGE fixed overhead is
    amortized), in bf16 (256B/row).
  - Per chunk: a val-weighted one-hot matrix oh[e, d] = (dloc[e]==d)*val[e]
    is built in one fused tensor_scalar (bf16, split between DVE and Pool
    engines), then matmul aggT[feat, dest] += gathered[e, feat].T @ oh[e, d]
    accumulates in PSUM (bf16 matmul: 1 cycle/row).
  - Per window: out[d, :] = (aggT.T @ W) + bias.  Aggregation happens in
    input-feature space so no support table and no collective is needed.
"""

import numpy as np
import ml_dtypes

N_NODES = 100000
N_EDGES = 1600000
IN_F = 128
OUT_F = 64
N_CORES = 8
P = 128
NR = 4                     # source ranges (int16 idx limit: rsz <= 32768)
WB = 8                     # dest windows per batch
MAXCH = 8                  # max chunks per gather piece (1024 idxs =
                           # single_packet + default-scratch SWDGE ring limit)
NQ = 4                     # SWDGE queues (descgen parallelizes across them)


def _cdiv(a, b):
    return -(-a // b)


def _preprocess(edge_row, edge_col, edge_val, n_nodes=N_NODES,
                n_cores=N_CORES, wb=WB, nr=NR):
    """Group edges by (core, window, src-range); equalize chunk counts across
    cores; build per-core idx/dloc/val tables and the shared program plan."""
    n_shard = n_nodes // n_cores
    n_win = _cdiv(n_shard, P)
    rsz = _cdiv(n_nodes, nr)
    nb = _cdiv(n_win, wb)

    rows = np.asarray(edge_row, np.int64)
    cols = np.asarray(edge_col, np.int64)
    vals = np.asarray(edge_val, np.float32)

    core = rows // n_shard
    local = rows - core * n_shard
    win = local // P
    dloc = (local % P).astype(np.float32)
    rng_ = cols // rsz
    idx16 = (cols - rng_ * rsz).astype(np.int16)

    cnt = np.zeros((n_cores, n_win, nr), np.int64)
    np.add.at(cnt, (core, win, rng_), 1)
    nch_wr = _cdiv(cnt.max(axis=0), P)             # [n_win, nr]
    empty_w = nch_wr.sum(axis=1) == 0
    nch_wr[empty_w, 0] = 1                         # every window needs >=1

    # enumerate chunks in (batch, range, window, k) order; build gather pieces
    cid_base = np.zeros((n_win, nr), np.int64)
    gathers = []                                   # (batch, range, cid0, nch)
    cid = 0
    for b in range(nb):
        ws = list(range(b * wb, min(n_win, (b + 1) * wb)))
        for r in range(nr):
            total = int(nch_wr[ws, r].sum())
            if total == 0:
                continue
            cc = cid
            for w in ws:
                cid_base[w, r] = cc
                cc += int(nch_wr[w, r])
            off = 0
            while off < total:
                n = min(MAXCH, total - off)
                gathers.append((b, r, cid + off, n))
                off += n
            cid += total
    nch_total = cid
    nidx = nch_total * P

    piece_of = {}
    for gi, (b, r, cb, n) in enumerate(gathers):
        for k in range(n):
            piece_of[cb + k] = (gi, k)
    windows = []                                   # [w] -> [(gi, slab, cid)]
    for w in range(n_win):
        lst = []
        for r in range(nr):
            for k in range(int(nch_wr[w, r])):
                c = int(cid_base[w, r]) + k
                gi, slab = piece_of[c]
                lst.append((gi, slab, c))
        windows.append(lst)

    # per-core tables
    idx_flat = np.zeros((n_cores, nidx), np.int16)
    dloc_tab = np.zeros((n_cores, P, nch_total), np.float32)
    vals_tab = np.zeros((n_cores, P, nch_total), np.float32)

    key = (core * n_win + win) * nr + rng_
    order = np.argsort(key, kind="stable")
    ks = key[order]
    starts = np.r_[0, np.nonzero(np.diff(ks))[0] + 1]
    gid = np.zeros(len(ks), np.int64)
    gid[starts[1:]] = 1
    gid = np.cumsum(gid)
    rank = np.arange(len(ks)) - starts[gid]
    ec, ew, er = core[order], win[order], rng_[order]
    chunk_k = rank // P
    part = rank % P
    cidv = cid_base[ew, er] + chunk_k
    dloc_tab[ec, part, cidv] = dloc[order]
    vals_tab[ec, part, cidv] = vals[order]
    idx_flat[ec, cidv * P + part] = idx16[order]

    wrapped = idx_flat.reshape(n_cores, nidx // 16, 16).transpose(0, 2, 1)
    idx_tab = np.ascontiguousarray(np.tile(wrapped, (1, 8, 1)))  # [c,128,cols]

    plan = dict(n_nodes=n_nodes, n_cores=n_cores, n_shard=n_shard,
                n_win=n_win, rsz=rsz, nb=nb, wb=wb, nr=nr,
                nch_total=nch_total, nidx=nidx,
                gathers=gathers, windows=windows)
    return plan, idx_tab, dloc_tab, vals_tab


def _build_nc(plan, in_f=IN_F, out_f=OUT_F):
    import concourse.bacc as bacc
    import concourse.mybir as mybir
    import concourse.tile as tile

    f32 = mybir.dt.float32
    bf16 = mybir.dt.bfloat16
    i16 = mybir.dt.int16
    nc = bacc.Bacc("TRN2", target_bir_lowering=False, debug=False,
                   enable_asserts=False, num_swdge_queues=NQ)

    n_nodes = plan["n_nodes"]
    n_shard = plan["n_shard"]
    n_win = plan["n_win"]
    rsz = plan["rsz"]
    nb = plan["nb"]
    wb = plan["wb"]
    nch_total = plan["nch_total"]
    nidx = plan["nidx"]
    gathers = plan["gathers"]
    windows = plan["windows"]

    maxnch = max(g[3] for g in gathers)
    pieces_by_batch = [[] for _ in range(nb)]
    for gi, g in enumerate(gathers):
        pieces_by_batch[g[0]].append(gi)
    gbufs = max(len(pieces_by_batch[b]) + len(pieces_by_batch[b + 1])
                for b in range(nb - 1)) if nb > 1 else len(pieces_by_batch[0])

    vertex = nc.dram_tensor("vertex", [n_nodes, in_f], bf16,
                            kind="ExternalInput").ap()
    wmat = nc.dram_tensor("wmat", [in_f, out_f], bf16,
                          kind="ExternalInput").ap()
    bias_rep = nc.dram_tensor("bias_rep", [P, out_f], bf16,
                              kind="ExternalInput").ap()
    ones = nc.dram_tensor("ones", [P, P], bf16, kind="ExternalInput").ap()
    iota = nc.dram_tensor("iota", [P, P], bf16, kind="ExternalInput").ap()
    idxs = nc.dram_tensor("idxs", [P, nidx // 16], i16,
                          kind="ExternalInput").ap()
    dloc = nc.dram_tensor("dloc", [P, nch_total], f32,
                          kind="ExternalInput").ap()
    vals = nc.dram_tensor("vals", [P, nch_total], f32,
                          kind="ExternalInput").ap()
    out = nc.dram_tensor("out", [n_shard, out_f], f32,
                         kind="ExternalOutput").ap()

    with tile.TileContext(nc) as tc:
        with (
            tc.tile_pool(name="const", bufs=1) as cpool,
            tc.tile_pool(name="meta", bufs=1) as mpool,
            tc.tile_pool(name="gather", bufs=gbufs) as gpool,
            tc.tile_pool(name="oh", bufs=12) as opool,
            tc.tile_pool(name="evac", bufs=4) as epool,
            tc.tile_pool(name="osb", bufs=4) as spool,
            tc.tile_pool(name="agg_psum", bufs=4, space="PSUM") as agg_pp,
            tc.tile_pool(name="out_psum", bufs=2, space="PSUM") as out_pp,
        ):
            w_sb = cpool.tile([in_f, out_f], bf16)
            nc.sync.dma_start(out=w_sb[:], in_=wmat[:])
            bias_sb = cpool.tile([P, out_f], bf16)
            nc.sync.dma_start(out=bias_sb[:], in_=bias_rep[:])
            ones_sb = cpool.tile([P, P], bf16)
            nc.sync.dma_start(out=ones_sb[:], in_=ones[:])
            iota_sb = cpool.tile([P, P], bf16)
            nc.sync.dma_start(out=iota_sb[:], in_=iota[:])
            idx_sb = mpool.tile([P, nidx // 16], i16)
            nc.sync.dma_start(out=idx_sb[:], in_=idxs[:])
            dloc_sb = mpool.tile([P, nch_total], f32)
            nc.sync.dma_start(out=dloc_sb[:], in_=dloc[:])
            vals_sb = mpool.tile([P, nch_total], f32)
            nc.sync.dma_start(out=vals_sb[:], in_=vals[:])

            tiles = {}

            def emit_gathers(b):
                for gi in pieces_by_batch[b]:
                    _, r, cb, n = gathers[gi]
                    t = gpool.tile([P, maxnch * in_f], bf16, tag="g")
                    nc.gpsimd.dma_gather(
                        out_ap=t[:, :n * in_f].rearrange(
                            "p (g f) -> p g f", f=in_f),
                        in_ap=vertex[r * rsz:n_nodes, :],
                        idxs_ap=idx_sb[:, cb * 8:(cb + n) * 8],
                        num_idxs=n * P,
                        num_idxs_reg=n * P,
                        elem_size=in_f,
                        single_packet=True,
                        queue_num=gi % NQ,
                    )
                    tiles[gi] = t

            emit_gathers(0)
            if nb > 1:
                emit_gathers(1)
            for b in range(nb):
                for w in range(b * wb, min(n_win, (b + 1) * wb)):
                    chunks = windows[w]
                    m = len(chunks)
                    aggT = agg_pp.tile([in_f, P], f32, tag="agg")
                    for j, (gi, slab, c) in enumerate(chunks):
                        oh = opool.tile([P, P], bf16, tag="oh")
                        nc.vector.tensor_scalar(
                            out=oh[:],
                            in0=iota_sb[:],
                            scalar1=dloc_sb[:, c:c + 1],
                            scalar2=vals_sb[:, c:c + 1],
                            op0=mybir.AluOpType.is_equal,
                            op1=mybir.AluOpType.mult,
                        )
                        nc.tensor.matmul(
                            out=aggT[:],
                            lhsT=tiles[gi][:, slab * in_f:(slab + 1) * in_f],
                            rhs=oh[:],
                            start=(j == 0),
                            stop=(j == m - 1),
                        )
                    aggT_sb = epool.tile([in_f, P], bf16, tag="ev")
                    nc.scalar.activation(
                        out=aggT_sb[:], in_=aggT[:],
                        func=mybir.ActivationFunctionType.Copy)
                    outw = out_pp.tile([P, out_f], f32, tag="ow")
                    # seed PSUM with bias via a K=1 matmul, then accumulate
                    nc.tensor.matmul(out=outw[:], lhsT=ones_sb[0:1, :],
                                     rhs=bias_sb[0:1, :], start=True,
                                     stop=False)
                    nc.tensor.matmul(out=outw[:], lhsT=aggT_sb[:],
                                     rhs=w_sb[:], start=False, stop=True)
                    out_sb = spool.tile([P, out_f], f32, tag="os")
                    nc.scalar.activation(
                        out=out_sb[:], in_=outw[:],
                        func=mybir.ActivationFunctionType.Copy)
                    rows = min(P, n_shard - w * P)
                    nc.sync.dma_start(out=out[w * P:w * P + rows, :],
                                      in_=out_sb[:rows, :])
                if b + 2 < nb:
                    emit_gathers(b + 2)

    nc.compile()
    return nc


def _make_in_maps(vertex, weights, bias, idx_tab, dloc_tab, vals_tab,
                  n_cores=N_CORES):
    iota = np.tile(np.arange(P, dtype=ml_dtypes.bfloat16)[None, :], (P, 1))
    bias_rep = np.ascontiguousarray(
        np.tile(np.asarray(bias, np.float32)[None, :],
                (P, 1)).astype(ml_dtypes.bfloat16))
    ones_t = np.ones((P, P), ml_dtypes.bfloat16)
    vertex_bf = np.ascontiguousarray(
        np.asarray(vertex, np.float32).astype(ml_dtypes.bfloat16))
    w_bf = np.ascontiguousarray(
        np.asarray(weights, np.float32).astype(ml_dtypes.bfloat16))
    return [
        {
            "vertex": vertex_bf,
            "wmat": w_bf,
            "bias_rep": bias_rep,
            "ones": ones_t,
            "iota": iota,
            "idxs": idx_tab[c],
            "dloc": dloc_tab[c],
            "vals": vals_tab[c],
        }
        for c in range(n_cores)
    ]


def _run(nc, in_maps, trace=False, tmpdir=None):
    from concourse import bass_utils
    from concourse.bass_interp import get_hw_module

    old_m = nc.m
    nc.m = get_hw_module(nc.m)
    try:
        return bass_utils.run_bass_kernel_spmd(
            nc, in_maps, core_ids=list(range(len(in_maps))),
            trace=trace, tmpdir=tmpdir)
    finally:
        nc.m = old_m


def kernel(**inputs):
    vertex = np.asarray(inputs["vertex"], dtype=np.float32)
    edge_row = np.asarray(inputs["edge_row"])
    edge_col = np.asarray(inputs["edge_col"])
    edge_val = np.asarray(inputs["edge_val"], dtype=np.float32)
    weights = np.asarray(inputs["weights"], dtype=np.float32)
    bias = np.asarray(inputs["bias"], dtype=np.float32)

    plan, idx_tab, dloc_tab, vals_tab = _preprocess(
        edge_row, edge_col, edge_val)
    nc = _build_nc(plan)
    in_maps = _make_in_maps(vertex, weights, bias, idx_tab, dloc_tab,
                            vals_tab)
    res = _run(nc, in_maps)
    return np.concatenate([res.results[c]["out"] for c in range(N_CORES)],
                          axis=0)
